# revision 5
# baseline (speedup 1.0000x reference)
"""Trainium2 Bass kernel for a transformer decoder layer (self-attn + cross-attn + FFN).

Sharding: 8-way tensor parallel over heads for both attentions (2 heads/core),
token-sharded (512 tokens/core) for the wo projections, layernorms and FFN.
Head<->token redistribution is done with three 8-core AllToAll collectives
(self-attn out, cross-attn q, cross-attn out); there are no all-reduces.

All matmuls run in bf16 with fp32 PSUM accumulation. Attention keeps the
[feature, token] (transposed) layout throughout: scoresT uses kT-chunk
stationary x qT moving, probs come out as PT[ki, qi] which feeds attnV
directly with V-natural (+ones column) stationary, producing attn^T and the
softmax denominator in one accumulation group. Normalization happens via a
reciprocal row broadcast with a rank-1 fp32r matmul.
"""

import sys

TRN_REPO = "/opt/trn_rl_repo"
if TRN_REPO not in sys.path:
    sys.path.insert(0, TRN_REPO)

import numpy as np
import ml_dtypes

D_MODEL = 1024
N_HEADS = 16
DFF = 4096
B, S = 2, 2048
EPS = 1e-6
DEPTH = D_MODEL // N_HEADS  # 64

NCORES = 8
HPC = N_HEADS // NCORES     # heads per core = 2
TOK = B * S                 # 4096 flattened tokens
TOWN = TOK // NCORES        # 512 tokens per core
KD = D_MODEL // 128         # 8 contraction chunks over d_model
FC = DFF // 128             # 32 chunks over dff
NBT = S // 512              # 4 q-tiles per batch
NBC = S // 128              # 16 ki-chunks per batch

BF = ml_dtypes.bfloat16

_PROG_CACHE = {}


def _build_program(self_blocks, n_ctiles):
    """Emit the SPMD Bass program (same program on all 8 cores).

    self_blocks: dict (t, c) -> 'full' | ('tile', idx) for allowed self-attn
                 blocks (skipped blocks absent), shared by both batches.
    n_ctiles:    number of unique partial-mask tiles in the `cmask` input.
    """
    import concourse.bacc as bacc
    import concourse.mybir as mybir
    from concourse import tile

    F32 = mybir.dt.float32
    F32R = mybir.dt.float32r
    BF16 = mybir.dt.bfloat16
    EXP = mybir.ActivationFunctionType.Exp
    IDENT = mybir.ActivationFunctionType.Identity
    LNF = mybir.ActivationFunctionType.Ln
    ADD = mybir.AluOpType.add
    MULT = mybir.AluOpType.mult
    SUB = mybir.AluOpType.subtract

    nc = bacc.Bacc("TRN2", target_bir_lowering=False, debug=False,
                   num_devices=NCORES)

    def din(name, shape, dt=BF16):
        return nc.dram_tensor(name, shape, dt, kind="ExternalInput")

    xT_d = din("xT", [D_MODEL, TOK])
    encT_d = din("encT", [D_MODEL, TOK])
    xown_d = din("x_own", [TOWN, D_MODEL], F32)
    wq1_d = din("wq1", [128, KD * 128])
    wk1_d = din("wk1", [128, KD * 128])
    wv1_d = din("wv1", [128, KD * 128])
    bq1_d = din("bq1", [128, 1], F32)
    bk1_d = din("bk1", [128, 1], F32)
    bv1_d = din("bv1", [128, 1], F32)
    wo1_d = din("wo1", [128, KD * 1024])
    bo1_d = din("bo1", [1, 1024])
    wq2_d = din("wq2", [128, KD * KD * 128])
    bq2_d = din("bq2", [128, KD], F32)
    wk2_d = din("wk2", [128, KD * 128])
    wv2_d = din("wv2", [128, KD * 128])
    bk2_d = din("bk2", [128, 1], F32)
    bv2_d = din("bv2", [128, 1], F32)
    wo2_d = din("wo2", [128, KD * 1024])
    bo2_d = din("bo2", [1, 1024])
    w1_d = din("w1", [128, FC * KD * 128])
    b1_d = din("b1", [1, DFF])
    w2_d = din("w2", [128, FC * 1024])
    b2_d = din("b2", [1, 1024])
    cm_d = din("cmask", [128, max(n_ctiles, 1) * 512])
    mb_d = din("mbias", [128, B * NBC], F32)
    out_d = nc.dram_tensor("out", [TOWN, D_MODEL], F32, kind="ExternalOutput")

    CROSS_BLOCKS = {(t, c): 'full' for t in range(NBT) for c in range(NBC)}
    GROUPS = [list(range(NCORES))]

    with tile.TileContext(nc) as tc:
      with tc.tile_pool(name="const", bufs=1) as constp, \
           tc.tile_pool(name="fbuf", bufs=1) as fbuf, \
           tc.tile_pool(name="lnsmall", bufs=2) as lns, \
           tc.tile_pool(name="dram", bufs=1, space="DRAM") as dram, \
           tc.tile_pool(name="ps_big", bufs=2, space="PSUM") as ps_big:

        # ---- constants ----
        ones65 = constp.tile([1, 65], F32)
        nc.vector.memset(ones65[:], 1.0)
        ones128b = constp.tile([1, 128], BF16)
        nc.vector.memset(ones128b[:], 1.0)
        ones512b = constp.tile([1, 512], BF16)
        nc.vector.memset(ones512b[:], 1.0)
        cm = constp.tile([128, max(n_ctiles, 1) * 512], BF16)
        nc.sync.dma_start(out=cm[:], in_=cm_d[:])
        mb = constp.tile([128, B * NBC], F32)
        nc.sync.dma_start(out=mb[:], in_=mb_d[:])

        # ---- persistent fp32/bf16 activations ----
        out1 = fbuf.tile([128, 4 * 1024], F32, tag="out1")
        out2 = fbuf.tile([128, 4 * 1024], F32, tag="out2")
        outT_a = fbuf.tile([128, KD * 512], BF16, tag="outT")  # out1T

        # ---- a2a dram buffers ----
        a2a1_in = dram.tile([NCORES * 128, TOWN], BF16)
        a2a1_out = dram.tile([NCORES * 128, TOWN], BF16)
        a2aq_in = dram.tile([NCORES * 128, TOWN], BF16)
        a2aq_out = dram.tile([NCORES * 128, TOWN], BF16)
        a2a2_in = dram.tile([NCORES * 128, TOWN], BF16)
        a2a2_out = dram.tile([NCORES * 128, TOWN], BF16)

        # ---------------- shared helpers ----------------
        def proj_transposed(pool, pspool, dst, w_sb, bias, src_sb):
            # dst[:, 512j:...] = (w_chunk^T @ srcT)[dcol, tok] + bias
            for j in range(TOK // 512):
                ps = pspool.tile([128, 512], F32, tag="psproj")
                for k in range(KD):
                    nc.tensor.matmul(
                        ps[:],
                        lhsT=w_sb[:, 128 * k:128 * (k + 1)],
                        rhs=src_sb[:, k * TOK + 512 * j:k * TOK + 512 * j + 512],
                        start=(k == 0), stop=(k == KD - 1))
                nc.scalar.activation(dst[:, 512 * j:512 * (j + 1)], ps[:],
                                     IDENT, bias=bias[:])

        def make_vaug(vT_sb, vt_tmp, vaug_sb):
            # vT_sb [128 (h,d), TOK] -> transpose chunks -> re-pack with
            # ones column: vaug[p, 65*(NBC*(B*h + b) + c) + d]
            for cg in range(TOK // 128):
                nc.sync.dma_start(out=vt_tmp[:, 128 * cg:128 * (cg + 1)],
                                  in_=vT_sb[:, 128 * cg:128 * (cg + 1)],
                                  transpose=True)
            nc.vector.memset(vaug_sb[:], 1.0)
            src = vt_tmp.rearrange("p (b c h d) -> p h b c d", b=B, c=NBC, h=HPC)
            dst = vaug_sb.rearrange("p (h b c d) -> p h b c d", h=HPC, b=B, c=NBC)
            nc.vector.tensor_copy(dst[:, :, :, :, 0:64], src[:])

        def vaug_slice(vaug_sb, h, b, c):
            base = 65 * (NBC * (B * h + b) + c)
            return vaug_sb[:, base:base + 65]

        def attention(pools, QT_sb, KT_sb, vaug_sb, stage_sb, blocks, bias_of):
            ps_s, ps_o, ps_b, ptp, smalls = pools
            for h in range(HPC):
                for b in range(B):
                    for t in range(NBT):
                        clist = [c for c in range(NBC) if (t, c) in blocks]
                        po = ps_o.tile([65, 512], F32, tag="po")
                        for ci, c in enumerate(clist):
                            kind = blocks[(t, c)]
                            ps = ps_s.tile([128, 512], F32, tag="ps")
                            nc.tensor.matmul(
                                ps[:],
                                lhsT=KT_sb[64 * h:64 * (h + 1),
                                           2048 * b + 128 * c:2048 * b + 128 * c + 128],
                                rhs=QT_sb[64 * h:64 * (h + 1),
                                          2048 * b + 512 * t:2048 * b + 512 * t + 512],
                                start=True, stop=True)
                            pt = ptp.tile([128, 512], BF16, tag="pt")
                            bias = bias_of(b, c)
                            if kind == 'full':
                                nc.scalar.activation(pt[:], ps[:], EXP,
                                                     scale=0.125, bias=bias)
                            else:
                                praw = ptp.tile([128, 512], BF16, tag="praw")
                                nc.scalar.activation(praw[:], ps[:], EXP,
                                                     scale=0.125, bias=bias)
                                idx = kind[1]
                                nc.vector.tensor_tensor(
                                    pt[:], praw[:],
                                    cm[:, 512 * idx:512 * (idx + 1)], op=MULT)
                            nc.tensor.matmul(
                                po[:], lhsT=vaug_slice(vaug_sb, h, b, c),
                                rhs=pt[:],
                                start=(ci == 0), stop=(ci == len(clist) - 1))
                        # normalize: out[d, qi] / denom[qi]
                        recip = smalls.tile([1, 512], F32R, tag="recip")
                        with nc.allow_low_precision(reason="softmax recip row"):
                            nc.vector.reciprocal(recip[:], po[64:65, :])
                        pb = ps_b.tile([65, 512], F32, tag="pb")
                        nc.tensor.matmul(pb[:], lhsT=ones65[:].bitcast(F32R),
                                         rhs=recip[:], start=True, stop=True)
                        pbsb = smalls.tile([64, 512], F32, tag="pbsb")
                        nc.scalar.copy(pbsb[:], pb[0:64, :])
                        nc.vector.tensor_tensor(
                            stage_sb[0:64,
                                     h * TOK + 2048 * b + 512 * t:
                                     h * TOK + 2048 * b + 512 * t + 512],
                            po[0:64, :], pbsb[:], op=MULT)

        def stage_to_a2a(stage_sb, a2a_in_t):
            for h in range(HPC):
                o = a2a_in_t.rearrange("(j r) s -> r j s", r=128)
                nc.sync.dma_start(
                    out=o[64 * h:64 * (h + 1)],
                    in_=stage_sb.rearrange("r (h j s) -> r h j s",
                                           h=HPC, j=NCORES)[:, h])

        def a2a(in_t, out_t):
            nc.gpsimd.collective_compute(
                "AllToAll", mybir.AluOpType.bypass, replica_groups=GROUPS,
                ins=[in_t.opt()], outs=[out_t.opt()])

        def ln_inplace(smalls, pre, dst):
            bnst = smalls.tile([128, 12], F32, tag="bnst")
            nc.vector.bn_stats(bnst[:, 0:6], pre[:, 0:512])
            nc.vector.bn_stats(bnst[:, 6:12], pre[:, 512:1024])
            stats = smalls.tile([128, 2], F32, tag="stats")
            nc.vector.bn_aggr(stats[:], bnst[:])
            veps = smalls.tile([128, 1], F32, tag="veps")
            nc.vector.tensor_scalar_add(veps[:], stats[:, 1:2], EPS)
            lnv = smalls.tile([128, 1], F32, tag="lnv")
            nc.scalar.activation(lnv[:], veps[:], LNF)
            rstd = smalls.tile([128, 1], F32, tag="rstd")
            nc.scalar.activation(rstd[:], lnv[:], EXP, scale=-0.5)
            nc.vector.tensor_scalar(dst[:], pre[:], stats[:, 0:1], rstd[:],
                                    op0=SUB, op1=MULT)

        def wo_ln_block(pool, at_sb, wo_sb, bo_sb, resid_of, outf, outT_sb):
            # outf[:, m*1024: ...] = LN(resid + at^T @ wo + bo), per m-tile
            for m in range(4):
                pre = pool.tile([128, 1024], F32, tag="pre")
                resid = resid_of(m)
                for eh in range(2):
                    pw = ps_big.tile([128, 512], F32, tag="psproj")
                    nc.tensor.matmul(pw[:], lhsT=ones128b[:],
                                     rhs=bo_sb[:, 512 * eh:512 * (eh + 1)],
                                     start=True, stop=False)
                    for dc in range(KD):
                        nc.tensor.matmul(
                            pw[:],
                            lhsT=at_sb[:, 512 * dc + 128 * m:
                                       512 * dc + 128 * m + 128],
                            rhs=wo_sb[:, 1024 * dc + 512 * eh:
                                      1024 * dc + 512 * eh + 512],
                            start=False, stop=(dc == KD - 1))
                    nc.vector.tensor_tensor(
                        pre[:, 512 * eh:512 * (eh + 1)], pw[:],
                        resid[:, 512 * eh:512 * (eh + 1)], op=ADD)
                ln_inplace(lns, pre, outf.rearrange("p (m e) -> p m e", m=4)[:, m])
                # bf16 copy + transpose into outT
                obf = pool.tile([128, 1024], BF16, tag="obf")
                nc.vector.tensor_copy(
                    obf[:], outf.rearrange("p (m e) -> p m e", m=4)[:, m])
                for j in range(KD):
                    nc.sync.dma_start(
                        out=outT_sb[:, 512 * j + 128 * m:512 * j + 128 * m + 128],
                        in_=obf[:, 128 * j:128 * (j + 1)], transpose=True)

        # ================= phases 1-2: self attention =====================
        with tc.tile_pool(name="p12", bufs=1) as p12, \
             tc.tile_pool(name="p12s", bufs=2) as p12s, \
             tc.tile_pool(name="pt12", bufs=3) as pt12, \
             tc.tile_pool(name="ps_s1", bufs=3, space="PSUM") as ps_s1, \
             tc.tile_pool(name="ps_o1", bufs=2, space="PSUM") as ps_o1, \
             tc.tile_pool(name="ps_b1", bufs=1, space="PSUM") as ps_b1:
            xT = p12.tile([128, KD * TOK], BF16, tag="xT")
            nc.sync.dma_start(
                out=xT.rearrange("p (k t) -> p k t", k=KD),
                in_=xT_d.rearrange("(k p) t -> p k t", p=128))
            wq1 = p12.tile([128, KD * 128], BF16, tag="wq1")
            wk1 = p12.tile([128, KD * 128], BF16, tag="wk1")
            wv1 = p12.tile([128, KD * 128], BF16, tag="wv1")
            nc.sync.dma_start(out=wq1[:], in_=wq1_d[:])
            nc.sync.dma_start(out=wk1[:], in_=wk1_d[:])
            nc.sync.dma_start(out=wv1[:], in_=wv1_d[:])
            bq1 = p12.tile([128, 1], F32, tag="bq1")
            bk1 = p12.tile([128, 1], F32, tag="bk1")
            bv1 = p12.tile([128, 1], F32, tag="bv1")
            nc.sync.dma_start(out=bq1[:], in_=bq1_d[:])
            nc.sync.dma_start(out=bk1[:], in_=bk1_d[:])
            nc.sync.dma_start(out=bv1[:], in_=bv1_d[:])

            QT = p12.tile([128, TOK], BF16, tag="QT")
            KT = p12.tile([128, TOK], BF16, tag="KT")
            # vT shares the (larger) stage slot: vT is consumed by make_vaug
            # before the attention writes stage1.
            vT1 = p12.tile([128, TOK], BF16, tag="stage")
            vt_tmp = p12.tile([128, TOK], BF16, tag="vt_tmp")
            vaug1 = p12.tile([128, HPC * B * NBC * 65], BF16, tag="vaug")

            proj_transposed(p12, ps_big, QT, wq1, bq1, xT)
            proj_transposed(p12, ps_big, KT, wk1, bk1, xT)
            proj_transposed(p12, ps_big, vT1, wv1, bv1, xT)
            make_vaug(vT1, vt_tmp, vaug1)

            stage1 = p12.tile([64, HPC * TOK], BF16, tag="stage")
            attention((ps_s1, ps_o1, ps_b1, pt12, p12s),
                      QT, KT, vaug1, stage1, self_blocks, lambda b, c: 0.0)
            stage_to_a2a(stage1, a2a1_in)

        a2a(a2a1_in, a2a1_out)

        # ====== phase 3 pool: cross K/V (+ QT2 slot), lives through attn2 ==
        with tc.tile_pool(name="p3", bufs=1) as p3:
            encT = p3.tile([128, KD * TOK], BF16, tag="encT")
            nc.sync.dma_start(
                out=encT.rearrange("p (k t) -> p k t", k=KD),
                in_=encT_d.rearrange("(k p) t -> p k t", p=128))
            wk2 = p3.tile([128, KD * 128], BF16, tag="wk2")
            wv2 = p3.tile([128, KD * 128], BF16, tag="wv2")
            nc.sync.dma_start(out=wk2[:], in_=wk2_d[:])
            nc.sync.dma_start(out=wv2[:], in_=wv2_d[:])
            bk2 = p3.tile([128, 1], F32, tag="bk2")
            bv2 = p3.tile([128, 1], F32, tag="bv2")
            nc.sync.dma_start(out=bk2[:], in_=bk2_d[:])
            nc.sync.dma_start(out=bv2[:], in_=bv2_d[:])

            KT2 = p3.tile([128, TOK], BF16, tag="KT2")
            vT2 = p3.tile([128, TOK], BF16, tag="vT2")
            vaug2 = p3.tile([128, HPC * B * NBC * 65], BF16, tag="vaug2")
            proj_transposed(p3, ps_big, KT2, wk2, bk2, encT)
            proj_transposed(p3, ps_big, vT2, wv2, bv2, encT)
            # vt_tmp2 shares a slot with QT2 (vt_tmp2 dies before QT2 arrives)
            vt_tmp2 = p3.tile([128, TOK], BF16, tag="qt2slot")
            make_vaug(vT2, vt_tmp2, vaug2)

            # ============ phase 4: wo1 + residual + LN1 + transpose =========
            with tc.tile_pool(name="p4", bufs=1) as p4, \
                 tc.tile_pool(name="p4s", bufs=2) as p4s:
                at1 = p4.tile([128, KD * 512], BF16, tag="at1")
                nc.sync.dma_start(
                    out=at1.rearrange("p (dc s) -> p dc s", dc=KD),
                    in_=a2a1_out.rearrange("(dc p) s -> p dc s", p=128))
                wo1 = p4.tile([128, KD * 1024], BF16, tag="wo1")
                nc.sync.dma_start(out=wo1[:], in_=wo1_d[:])
                bo1 = p4.tile([1, 1024], BF16, tag="bo1")
                nc.sync.dma_start(out=bo1[:], in_=bo1_d[:])

                def xown_resid(m):
                    xt = p4s.tile([128, 1024], F32, tag="xstream")
                    nc.sync.dma_start(out=xt[:],
                                      in_=xown_d[128 * m:128 * (m + 1), :])
                    return xt

                wo_ln_block(p4s, at1, wo1, bo1, xown_resid, out1, outT_a)

            # ============ phase 5: cross q projection + a2a =================
            with tc.tile_pool(name="p5", bufs=1) as p5:
                wq2 = p5.tile([128, KD * KD * 128], BF16, tag="wq2")
                nc.sync.dma_start(out=wq2[:], in_=wq2_d[:])
                bq2 = p5.tile([128, KD], F32, tag="bq2")
                nc.sync.dma_start(out=bq2[:], in_=bq2_d[:])
                qt2 = p5.tile([128, KD * 512], BF16, tag="qt2")
                for j in range(KD):
                    pq = ps_big.tile([128, 512], F32, tag="psproj")
                    for k in range(KD):
                        nc.tensor.matmul(
                            pq[:],
                            lhsT=wq2[:, 1024 * j + 128 * k:
                                     1024 * j + 128 * k + 128],
                            rhs=outT_a[:, 512 * k:512 * (k + 1)],
                            start=(k == 0), stop=(k == KD - 1))
                    nc.scalar.activation(qt2[:, 512 * j:512 * (j + 1)], pq[:],
                                         IDENT, bias=bq2[:, j:j + 1])
                nc.sync.dma_start(
                    out=a2aq_in.rearrange("(j p) s -> p j s", p=128),
                    in_=qt2.rearrange("p (j s) -> p j s", j=KD))
            a2a(a2aq_in, a2aq_out)
            QT2 = p3.tile([128, TOK], BF16, tag="qt2slot")
            nc.sync.dma_start(
                out=QT2.rearrange("p (i s) -> p i s", i=NCORES),
                in_=a2aq_out.rearrange("(i p) s -> p i s", p=128))

            # ============ phase 6: cross attention -> a2a2 ==================
            with tc.tile_pool(name="p6", bufs=1) as p6, \
                 tc.tile_pool(name="p6s", bufs=2) as p6s, \
                 tc.tile_pool(name="pt6", bufs=3) as pt6, \
                 tc.tile_pool(name="ps_s2", bufs=3, space="PSUM") as ps_s2, \
                 tc.tile_pool(name="ps_o2", bufs=2, space="PSUM") as ps_o2, \
                 tc.tile_pool(name="ps_b2", bufs=1, space="PSUM") as ps_b2:
                stage2 = p6.tile([64, HPC * TOK], BF16, tag="stage2")
                attention((ps_s2, ps_o2, ps_b2, pt6, p6s),
                          QT2, KT2, vaug2, stage2, CROSS_BLOCKS,
                          lambda b, c: mb[:, NBC * b + c:NBC * b + c + 1])
                stage_to_a2a(stage2, a2a2_in)
            a2a(a2a2_in, a2a2_out)

        # ============ phases 7-8: wo2 + LN2 + FFN + LN3 =====================
        with tc.tile_pool(name="p78", bufs=1) as p78, \
             tc.tile_pool(name="p78s", bufs=2) as p78s, \
             tc.tile_pool(name="w1stream", bufs=3) as w1s_pool:
            at2 = p78.tile([128, KD * 512], BF16, tag="at2")
            nc.sync.dma_start(
                out=at2.rearrange("p (dc s) -> p dc s", dc=KD),
                in_=a2a2_out.rearrange("(dc p) s -> p dc s", p=128))
            wo2 = p78.tile([128, KD * 1024], BF16, tag="wo2")
            nc.sync.dma_start(out=wo2[:], in_=wo2_d[:])
            bo2 = p78.tile([1, 1024], BF16, tag="bo2")
            nc.sync.dma_start(out=bo2[:], in_=bo2_d[:])
            # out2T reuses the out1T slot (out1T dead after phase 5)
            outT_b = fbuf.tile([128, KD * 512], BF16, tag="outT")
            out1v = out1.rearrange("p (m e) -> p m e", m=4)
            wo_ln_block(p78s, at2, wo2, bo2, lambda m: out1v[:, m],
                        out2, outT_b)

            b1 = p78.tile([1, DFF], BF16, tag="b1")
            nc.sync.dma_start(out=b1[:], in_=b1_d[:])
            b2 = p78.tile([1, 1024], BF16, tag="b2")
            nc.sync.dma_start(out=b2[:], in_=b2_d[:])
            hT = p78.tile([128, FC * 512], BF16, tag="hT")
            for fc in range(FC):
                w1t = w1s_pool.tile([128, KD * 128], BF16, tag="w1s")
                nc.sync.dma_start(out=w1t[:],
                                  in_=w1_d[:, 1024 * fc:1024 * (fc + 1)])
                ph = ps_big.tile([128, 512], F32, tag="psproj")
                nc.tensor.matmul(ph[:], lhsT=b1[:, 128 * fc:128 * (fc + 1)],
                                 rhs=ones512b[:], start=True, stop=False)
                for k in range(KD):
                    nc.tensor.matmul(ph[:],
                                     lhsT=w1t[:, 128 * k:128 * (k + 1)],
                                     rhs=outT_b[:, 512 * k:512 * (k + 1)],
                                     start=False, stop=(k == KD - 1))
                nc.vector.tensor_scalar_max(hT[:, 512 * fc:512 * (fc + 1)],
                                            ph[:], 0.0)

            w2 = p78.tile([128, FC * 1024], BF16, tag="w2")
            nc.sync.dma_start(out=w2[:], in_=w2_d[:])
            out2v = out2.rearrange("p (m e) -> p m e", m=4)
            for m in range(4):
                pre = p78s.tile([128, 1024], F32, tag="pre")
                for eh in range(2):
                    py = ps_big.tile([128, 512], F32, tag="psproj")
                    nc.tensor.matmul(py[:], lhsT=ones128b[:],
                                     rhs=b2[:, 512 * eh:512 * (eh + 1)],
                                     start=True, stop=False)
                    for fc in range(FC):
                        nc.tensor.matmul(
                            py[:],
                            lhsT=hT[:, 512 * fc + 128 * m:
                                    512 * fc + 128 * m + 128],
                            rhs=w2[:, 1024 * fc + 512 * eh:
                                   1024 * fc + 512 * eh + 512],
                            start=False, stop=(fc == FC - 1))
                    nc.vector.tensor_tensor(
                        pre[:, 512 * eh:512 * (eh + 1)], py[:],
                        out2v[:, m, 512 * eh:512 * (eh + 1)], op=ADD)
                outf = p78s.tile([128, 1024], F32, tag="outf")
                ln_inplace(lns, pre, outf)
                nc.sync.dma_start(out=out_d[128 * m:128 * (m + 1), :],
                                  in_=outf[:])

    nc.compile()
    return nc


def _to_bf(a):
    return np.ascontiguousarray(np.asarray(a, np.float32).astype(BF))


def _rechunk_k(w):
    """[K*128, M] -> [128, K*M] with col k*M + m = w[k*128 + p, m]."""
    K = w.shape[0] // 128
    M = w.shape[1]
    return np.ascontiguousarray(
        w.reshape(K, 128, M).transpose(1, 0, 2).reshape(128, K * M))


def _analyze_self_mask(mask):
    """mask [S, S] (1 = disallowed), orientation [q, k].

    Returns blocks dict (t, c) -> 'full' | ('tile', idx), list of unique
    multiplicative tiles [128, 512] (bf16), for a block grid over one batch.
    Blocks where everything is disallowed are omitted.
    """
    add = np.float32(-1e9) * np.asarray(mask, np.float32)
    mult = np.exp(add.T)  # [k, q] multiplicative
    blocks = {}
    tiles = []
    tile_ids = {}
    for t in range(NBT):
        for c in range(NBC):
            sub = mult[128 * c:128 * (c + 1), 512 * t:512 * (t + 1)]
            if not sub.any():
                continue
            if (sub == 1.0).all():
                blocks[(t, c)] = 'full'
                continue
            key = sub.tobytes()
            if key not in tile_ids:
                tile_ids[key] = len(tiles)
                tiles.append(sub.astype(BF))
            blocks[(t, c)] = ('tile', tile_ids[key])
    return blocks, tiles


def kernel(**inputs):
    from concourse.bass_utils import run_bass_kernel_spmd

    x = np.asarray(inputs["x"], np.float32)
    enc = np.asarray(inputs["enc_output"], np.float32)
    lam = np.asarray(inputs["look_ahead_mask"], np.float32)[0, 0]
    pad = np.asarray(inputs["padding_mask"], np.float32)  # [B,1,1,S]

    self_blocks, ctiles = _analyze_self_mask(lam)
    n_ctiles = len(ctiles)
    key = (tuple(sorted(self_blocks.items())), n_ctiles)
    if key not in _PROG_CACHE:
        _PROG_CACHE[key] = _build_program(self_blocks, n_ctiles)
    nc = _PROG_CACHE[key]

    # ---- shared (core-independent) host prep ----
    xf = x.reshape(TOK, D_MODEL)             # flattened batch-major tokens
    encf = enc.reshape(TOK, D_MODEL)
    xT = _to_bf(xf.T)                        # [1024, 4096]
    encT = _to_bf(encf.T)
    if n_ctiles:
        cmask = np.concatenate(ctiles, axis=1)
    else:
        cmask = np.zeros((128, 512), BF)
    cmask = np.ascontiguousarray(cmask)
    # cross-attn additive bias per enc token: [128, B*16], col b*16+c
    mb = (np.float32(-1e9) * pad[:, 0, 0, :]).reshape(B, NBC, 128)
    mb = np.ascontiguousarray(mb.transpose(2, 0, 1).reshape(128, B * NBC)
                              ).astype(np.float32)

    w1f = np.asarray(inputs["ffn_w1"], np.float32)
    # w1 stationary layout: [128, fc*1024 + k*128 + m] = w1[k*128+p, fc*128+m]
    w1r = w1f.reshape(KD, 128, FC, 128).transpose(1, 2, 0, 3)
    w1r = _to_bf(w1r.reshape(128, FC * KD * 128))
    w2r = _to_bf(_rechunk_k(np.asarray(inputs["ffn_w2"], np.float32)))
    b1 = _to_bf(np.asarray(inputs["ffn_b1"], np.float32)[None, :])
    b2 = _to_bf(np.asarray(inputs["ffn_b2"], np.float32)[None, :])

    wo1r = _to_bf(_rechunk_k(np.asarray(inputs["mha1_wo"], np.float32)))
    wo2r = _to_bf(_rechunk_k(np.asarray(inputs["mha2_wo"], np.float32)))
    bo1 = _to_bf(np.asarray(inputs["mha1_bo"], np.float32)[None, :])
    bo2 = _to_bf(np.asarray(inputs["mha2_bo"], np.float32)[None, :])

    wq2_full = np.asarray(inputs["mha2_wq"], np.float32)
    # wq2 stationary layout: [128, j*1024 + k*128 + m] = wq2[k*128+p, j*128+m]
    wq2r = wq2_full.reshape(KD, 128, KD, 128).transpose(1, 2, 0, 3)
    wq2r = _to_bf(wq2r.reshape(128, KD * KD * 128))
    bq2 = np.asarray(inputs["mha2_bq"], np.float32).reshape(KD, 128)
    bq2 = np.ascontiguousarray(bq2.T).astype(np.float32)  # [128, KD]

    in_maps = []
    for j in range(NCORES):
        hs = slice(128 * j, 128 * (j + 1))       # this core's 2 heads' cols
        m = {
            "xT": xT, "encT": encT,
            "x_own": np.ascontiguousarray(xf[TOWN * j:TOWN * (j + 1)]),
            "cmask": cmask, "mbias": mb,
            "w1": w1r, "b1": b1, "w2": w2r, "b2": b2,
            "wo1": wo1r, "bo1": bo1, "wo2": wo2r, "bo2": bo2,
            "wq2": wq2r, "bq2": bq2,
        }
        for pre, name in (("wq1", "mha1_wq"), ("wk1", "mha1_wk"),
                          ("wv1", "mha1_wv"), ("wk2", "mha2_wk"),
                          ("wv2", "mha2_wv")):
            w = np.asarray(inputs[name], np.float32)[:, hs]
            m[pre] = _to_bf(_rechunk_k(w))
        for pre, name in (("bq1", "mha1_bq"), ("bk1", "mha1_bk"),
                          ("bv1", "mha1_bv"), ("bk2", "mha2_bk"),
                          ("bv2", "mha2_bv")):
            bvec = np.asarray(inputs[name], np.float32)[hs]
            m[pre] = np.ascontiguousarray(bvec[:, None])
        in_maps.append(m)

    res = run_bass_kernel_spmd(nc, in_maps, list(range(NCORES)))
    out = np.empty((TOK, D_MODEL), np.float32)
    for j in range(NCORES):
        out[TOWN * j:TOWN * (j + 1)] = res.results[j]["out"]
    return out.reshape(B, S, D_MODEL)
